# revision 1
# baseline (speedup 1.0000x reference)
"""AutoRec forward pass on 8 Trainium2 NeuronCores (SPMD, no collectives).

Computation (reference):
    z = segment_sum(r[:,None] * V[u], i, num_segments=m)   # (m, D) sparse spmm
    h = sigmoid(z + mu)
    out = sum(h[j] * W[v], -1) + b[v]                      # (n_out,)

Strategy:
  - Users range-sharded over the 8 cores (m/8 each); edges bucketed host-side
    by (core, 128-user tile) so each core owns its z/h rows -> no collectives.
  - Phase 1, per user tile: V[u] rows fetched 128/instruction via indirect
    DMA (one row per partition, the HW-supported form); the segment scatter is
    zT += Vg^T @ S on the PE, where S[e,s] = (i_e-tile_base==s)*r_e is built
    on DVE from an iota tile in one fused tensor_scalar op. mu enters the
    same PSUM group as a rank-1 matmul; sigmoid (ACT) writes a column slice
    of a transposed h buffer hT [128 d, users] that stays RESIDENT IN SBUF.
  - Phase 2: pairs sharded by user, grouped per 128-user window, sorted by v
    inside each window. Per 128-pair block: gather W[v] rows (the only DMA
    gather left), PE-transpose them, QT[k,s] = WgT^T @ hT_window on PE, then
    one tensor_tensor_reduce with the one-hot E[k,s]=(jrel_k==s) picks
    QT[k, jrel_k] per pair and folds in the b[v] bias as the reduce init.
  - Host does index bucketing/permutation + the tiny b[v] lookup table.
"""

import math
import sys

sys.path.insert(0, "/opt/trn_rl_repo")

import numpy as np

D = 128
M_CORES = 8

_PROGRAM_CACHE: dict = {}


def _build_program(NI, T1, B, PB):
    """Build + compile the SPMD Bass program (identical on all cores).

    NI: item count; T1: user tiles/core; B: edge blocks per user tile;
    PB: pair blocks per user window.
    """
    import concourse.bacc as bacc
    import concourse.bass as bass
    import concourse.mybir as mybir
    from concourse.tile import TileContext

    f32 = mybir.dt.float32
    bf16 = mybir.dt.bfloat16
    i32 = mybir.dt.int32
    ALU = mybir.AluOpType
    ACT = mybir.ActivationFunctionType

    nc = bacc.Bacc("TRN2", target_bir_lowering=False, debug=False)

    V_d = nc.dram_tensor("V", [NI, D], f32, kind="ExternalInput")
    W_d = nc.dram_tensor("W", [NI, D], f32, kind="ExternalInput")
    mu_d = nc.dram_tensor("mu", [1, D], f32, kind="ExternalInput")
    eidx_d = nc.dram_tensor("eidx", [T1, 128, B], i32, kind="ExternalInput")
    erel_d = nc.dram_tensor("erel", [T1, 128, B], f32, kind="ExternalInput")
    erat_d = nc.dram_tensor("erat", [T1, 128, B], f32, kind="ExternalInput")
    pv_d = nc.dram_tensor("pv", [T1, 128, PB], i32, kind="ExternalInput")
    PB4 = ((PB + 3) // 4) * 4
    pjrelr_d = nc.dram_tensor("pjrelr", [T1, 1, PB4 * 128], f32, kind="ExternalInput")
    pb_d = nc.dram_tensor("pb", [T1, 128, PB], f32, kind="ExternalInput")
    res_d = nc.dram_tensor("res", [T1, 128, PB], f32, kind="ExternalOutput")

    with TileContext(nc) as tc:
        with tc.tile_pool(name="const", bufs=1) as constp:
            iota_i = constp.tile([128, 128], i32)
            nc.gpsimd.iota(iota_i[:], pattern=[[1, 128]], base=0, channel_multiplier=0)
            iota_f = constp.tile([128, 128], f32)
            nc.vector.tensor_copy(iota_f[:], iota_i[:])
            iotac_i = constp.tile([128, 1], i32)
            nc.gpsimd.iota(iotac_i[:], pattern=[[1, 1]], base=0, channel_multiplier=1)
            iotac_f = constp.tile([128, 1], f32)
            nc.vector.tensor_copy(iotac_f[:], iotac_i[:])
            ones1 = constp.tile([1, 128], f32)
            nc.vector.memset(ones1[:], 1.0)
            mu_t = constp.tile([1, 128], f32)
            nc.sync.dma_start(out=mu_t[:], in_=mu_d[:])
            hres = constp.tile([128, T1 * 128], f32)  # resident h: [user-in-tile, (t,d)]

            # -------- phase 1: hres[:, t] = sigmoid(S^T @ V[u] + mu) per user tile
            with tc.tile_pool(name="p1meta", bufs=3) as mp, \
                 tc.tile_pool(name="p1g", bufs=12) as gp, \
                 tc.tile_pool(name="p1s", bufs=6) as spool, \
                 tc.tile_pool(name="p1z", bufs=2, space="PSUM") as pp:
                for t in range(T1):
                    it = mp.tile([128, B], i32, tag="it")
                    nc.sync.dma_start(out=it[:], in_=eidx_d[t])
                    rel = mp.tile([128, B], f32, tag="rel")
                    nc.sync.dma_start(out=rel[:], in_=erel_d[t])
                    rat = mp.tile([128, B], f32, tag="rat")
                    nc.sync.dma_start(out=rat[:], in_=erat_d[t])
                    zt = pp.tile([128, 128], f32)
                    # z[s, d] = mu[d] (starts the accumulation group; fp32 exact)
                    nc.tensor.matmul(zt[:], lhsT=ones1[:], rhs=mu_t[:],
                                     start=True, stop=False)
                    for bb in range(B):
                        g = gp.tile([128, D], bf16, tag="g")
                        nc.gpsimd.indirect_dma_start(
                            out=g[:], out_offset=None, in_=V_d[:],
                            in_offset=bass.IndirectOffsetOnAxis(
                                ap=it[:, bb:bb + 1], axis=0))
                        S = spool.tile([128, 128], bf16, tag="S")
                        nc.vector.tensor_scalar(
                            out=S[:], in0=iota_f[:],
                            scalar1=rel[:, bb:bb + 1], scalar2=rat[:, bb:bb + 1],
                            op0=ALU.is_equal, op1=ALU.mult)
                        nc.tensor.matmul(zt[:], lhsT=S[:], rhs=g[:],
                                         start=False, stop=(bb == B - 1))
                    nc.scalar.activation(hres[:, t * 128:(t + 1) * 128], zt[:],
                                         ACT.Sigmoid)

            # -------- phase 2: res = b + sum_d (E^T @ h_window)[k,d] * W[v][k,d]
            with tc.tile_pool(name="p2meta", bufs=3) as mp2, \
                 tc.tile_pool(name="p2g", bufs=12) as gp2, \
                 tc.tile_pool(name="p2e", bufs=6) as ep, \
                 tc.tile_pool(name="p2sc", bufs=6) as scp, \
                 tc.tile_pool(name="p2r", bufs=3) as rp, \
                 tc.tile_pool(name="p2jb", bufs=2, space="PSUM") as pjb, \
                 tc.tile_pool(name="p2q", bufs=2, space="PSUM") as pq:
                for t in range(T1):
                    vt = mp2.tile([128, PB], i32, tag="vt")
                    nc.sync.dma_start(out=vt[:], in_=pv_d[t])
                    jrow = mp2.tile([1, PB4 * 128], f32, tag="jrow")
                    nc.sync.dma_start(out=jrow[:], in_=pjrelr_d[t])
                    pbt = mp2.tile([128, PB], f32, tag="pbt")
                    nc.sync.dma_start(out=pbt[:], in_=pb_d[t])
                    rt = rp.tile([128, PB], f32, tag="rt")
                    jb = None
                    for bb in range(PB):
                        wg = gp2.tile([128, D], f32, tag="wg")
                        nc.gpsimd.indirect_dma_start(
                            out=wg[:], out_offset=None, in_=W_d[:],
                            in_offset=bass.IndirectOffsetOnAxis(
                                ap=vt[:, bb:bb + 1], axis=0))
                        if bb % 4 == 0:
                            # JB[s, k] = jrel_k for every s, 4 blocks at a time
                            jb = pjb.tile([128, 512], f32, tag="jb")
                            nc.tensor.matmul(jb[:], lhsT=ones1[:],
                                             rhs=jrow[:, bb * 128:bb * 128 + 512],
                                             start=True, stop=True)
                        # E[s, k] = (jrel_k == s)
                        E = ep.tile([128, 128], f32, tag="E")
                        nc.vector.tensor_scalar(
                            out=E[:], in0=jb[:, (bb % 4) * 128:(bb % 4 + 1) * 128],
                            scalar1=iotac_f[:], scalar2=None,
                            op0=ALU.is_equal)
                        # Hsel[k, d] = h[jrel_k, d] for this window
                        hs = pq.tile([128, 128], f32)
                        nc.tensor.matmul(hs[:], lhsT=E[:],
                                         rhs=hres[:, t * 128:(t + 1) * 128],
                                         start=True, stop=True)
                        sc = scp.tile([128, 128], f32, tag="sc")
                        nc.vector.scalar_tensor_tensor(
                            out=sc[:], in0=hs[:], scalar=1.0, in1=wg[:],
                            op0=ALU.mult, op1=ALU.mult,
                            accum_out=rt[:, bb:bb + 1])
                    nc.vector.tensor_add(rt[:], rt[:], pbt[:])
                    nc.sync.dma_start(out=res_d[t], in_=rt[:])

    nc.compile()
    return nc


def _prep_inputs(u, i, r, m, v, j, V, mu, W, b):
    """Host-side sharding. Returns per-core input maps + unshard info."""
    NU = int(m)
    NI = int(V.shape[0])
    NOUT = int(v.shape[0])
    UC = (NU + M_CORES - 1) // M_CORES       # users per core
    T1 = (UC + 127) // 128                   # 128-user tiles per core

    u32 = np.asarray(u).astype(np.int32)
    i32 = np.asarray(i).astype(np.int32)
    r32 = np.asarray(r, dtype=np.float32)

    c_e = i32 // UC
    tloc = (i32 - c_e * UC) >> 7
    gtile = c_e * T1 + tloc
    irel = (i32 - c_e * UC - (tloc << 7)).astype(np.float32)
    order = np.lexsort((u32, gtile))
    gt_s = gtile[order]
    us = u32[order]
    rs = r32[order]
    irel_s = irel[order]

    NT = M_CORES * T1
    counts = np.bincount(gt_s, minlength=NT)
    B = max(1, int(math.ceil(counts.max() / 128)))
    starts = np.zeros(NT + 1, np.int64)
    np.cumsum(counts, out=starts[1:])

    eidx = np.zeros((M_CORES, T1, 128, B), np.int32)
    erel = np.zeros((M_CORES, T1, 128, B), np.float32)
    erat = np.zeros((M_CORES, T1, 128, B), np.float32)
    for gidx in range(NT):
        s, e = int(starts[gidx]), int(starts[gidx + 1])
        k = e - s
        if k == 0:
            continue
        c, t = divmod(gidx, T1)
        pu = np.zeros(B * 128, np.int32)
        pu[:k] = us[s:e]
        pr = np.zeros(B * 128, np.float32)
        pr[:k] = irel_s[s:e]
        pa = np.zeros(B * 128, np.float32)
        pa[:k] = rs[s:e]
        eidx[c, t] = pu.reshape(B, 128).T
        erel[c, t] = pr.reshape(B, 128).T
        erat[c, t] = pa.reshape(B, 128).T

    # ---- decode pairs: bucket by (core, 128-user window), sort by v inside
    j32 = np.asarray(j).astype(np.int32)
    v32 = np.asarray(v).astype(np.int32)
    bvec = np.asarray(b, dtype=np.float32).reshape(-1)
    cj = j32 // UC
    tj = (j32 - cj * UC) >> 7
    gwin = cj * T1 + tj
    jrel_all = (j32 - cj * UC - (tj << 7)).astype(np.float32)
    order2 = np.lexsort((v32, gwin))
    gw_s = gwin[order2]
    v_s = v32[order2]
    jr_s = jrel_all[order2]
    pb_s = bvec[v_s]

    counts2 = np.bincount(gw_s, minlength=NT)
    PB = max(1, int(math.ceil(counts2.max() / 128)))
    st2 = np.zeros(NT + 1, np.int64)
    np.cumsum(counts2, out=st2[1:])

    PB4 = ((PB + 3) // 4) * 4
    pv = np.zeros((M_CORES, T1, 128, PB), np.int32)
    pjrelr = np.zeros((M_CORES, T1, 1, PB4 * 128), np.float32)
    pb = np.zeros((M_CORES, T1, 128, PB), np.float32)
    for gidx in range(NT):
        s, e = int(st2[gidx]), int(st2[gidx + 1])
        k = e - s
        if k == 0:
            continue
        c, t = divmod(gidx, T1)
        a = np.zeros(PB * 128, np.int32)
        a[:k] = v_s[s:e]
        bbuf = np.zeros(PB * 128, np.float32)
        bbuf[:k] = jr_s[s:e]
        cbuf = np.zeros(PB * 128, np.float32)
        cbuf[:k] = pb_s[s:e]
        pv[c, t] = a.reshape(PB, 128).T
        pjrelr[c, t, 0, :PB * 128] = bbuf
        pb[c, t] = cbuf.reshape(PB, 128).T

    Vf = np.ascontiguousarray(V, dtype=np.float32)
    Wf = np.ascontiguousarray(W, dtype=np.float32)
    muf = np.ascontiguousarray(np.asarray(mu).reshape(1, D), dtype=np.float32)

    in_maps = []
    for c in range(M_CORES):
        in_maps.append({
            "V": Vf, "W": Wf, "mu": muf,
            "eidx": eidx[c], "erel": erel[c], "erat": erat[c],
            "pv": pv[c], "pjrelr": pjrelr[c], "pb": pb[c],
        })
    meta = dict(NI=NI, T1=T1, B=B, PB=PB, NOUT=NOUT,
                counts2=counts2, st2=st2, order2=order2)
    return in_maps, meta


def _unshard(results, meta):
    T1, PB = meta["T1"], meta["PB"]
    counts2 = meta["counts2"]
    order2 = meta["order2"]
    NT = M_CORES * T1
    parts = []
    for gidx in range(NT):
        c, t = divmod(gidx, T1)
        k = int(counts2[gidx])
        if k == 0:
            continue
        flat = results[c]["res"][t].T.reshape(-1)  # (p, bb) -> bb*128+p
        parts.append(flat[:k])
    out = np.empty(meta["NOUT"], np.float32)
    out[order2] = np.concatenate(parts) if parts else np.empty(0, np.float32)
    return out


def run(u, i, r, m, v, j, V, mu, W, b, trace=False, trace_kwargs=None):
    """Full pipeline; returns (out, BassKernelResults)."""
    from concourse import bass_utils

    in_maps, meta = _prep_inputs(u, i, r, m, v, j, V, mu, W, b)
    key = (meta["NI"], meta["T1"], meta["B"], meta["PB"])
    nc = _PROGRAM_CACHE.get(key)
    if nc is None:
        nc = _build_program(*key)
        _PROGRAM_CACHE[key] = nc
    res = bass_utils.run_bass_kernel_spmd(
        nc, in_maps, list(range(M_CORES)), trace=trace, **(trace_kwargs or {}))
    return _unshard(res.results, meta), res


def kernel(u, i, r, m, v, j, V, mu, W, b):
    out, _ = run(u, i, r, m, v, j, V, mu, W, b, trace=False)
    return out

